# revision 5
# baseline (speedup 1.0000x reference)
"""4-bit groupwise-quantized linear layer (CLinear) on 8 Trainium2 NeuronCores.

Full-input contract: kernel(**inputs) takes the unsharded numpy inputs
  x      [4, 2048, 4096] fp32
  packed [4096, 64, 32]  int32 (byte values; hi nibble = first half of each
                                quant group, lo nibble = second half)
  mn     [4096, 64, 1]   fp32
  scale  [4096, 64, 1]   fp32
  bias   [4096]          fp32
and returns out[4, 2048, 4096] fp32 = x @ dequant(packed, mn, scale).T + bias.

Sharding: 2D grid, 4 token-row groups x 2 out-column groups. Core (r, c)
computes out[r*2048:(r+1)*2048, c*2048:(c+1)*2048] from its x row-shard and
its packed/mn/scale/bias column-shard. No collectives; host concatenates.

Device kernel per core:
  - dequantize the 2048x4096 weight shard on-chip (DVE nibble extraction +
    scale/offset), bf16, and DMA-transpose it into a resident [k, n] SBUF
    layout;
  - stream x row-tiles: fp32->bf16 convert (scalar engine), DMA-transpose to
    [k, m] tiles, then bf16 matmuls accumulating fp32 in PSUM (bias folded in
    as a K=1 matmul);
  - copy PSUM->SBUF on the scalar engine and DMA out.
"""

import os
import sys
from contextlib import ExitStack

import numpy as np

if "/opt/trn_rl_repo" not in sys.path:
    sys.path.insert(0, "/opt/trn_rl_repo")

import concourse.bass as bass
import concourse.mybir as mybir
import concourse.tile as tile
from concourse import bacc
from concourse.bass_utils import run_bass_kernel_spmd

FP32 = mybir.dt.float32
BF16 = mybir.dt.bfloat16
I32 = mybir.dt.int32
P = 128
GS = 64  # quant group size

# problem shape (hardcoded)
B, S, IN, OUT = 4, 2048, 4096, 4096
R_SHARDS, C_SHARDS = 4, 2  # token-row x out-column grid over 8 cores
M_CORE = B * S // R_SHARDS      # 2048 tokens per core
N_CORE = OUT // C_SHARDS        # 2048 out features per core
NB = 512                        # psum n-block


def _emit_kernel(tc, outs, ins, M, K, N, NB=512):
    nc = tc.nc
    ctx = ExitStack()
    G = K // GS          # quant groups along K
    KT = K // P          # k-tiles
    NT = N // P          # weight row-tiles
    MT = M // P          # x row-tiles
    NBS = N // NB        # psum n-blocks
    G_CH = min(16, G)    # groups per dequant chunk (16 groups = 1024 k)
    GC = G // G_CH       # dequant chunks per row-tile
    K_CH = G_CH * GS     # k elements per dequant chunk
    X_CH = min(1024, K)  # x fp32 load chunk (free elems)

    x_d = ins["x"]            # [M, K] fp32
    pk_d = ins["packed"]      # [N, G, 32] int32
    mn_d = ins["mn"]          # [N, G] fp32
    sc_d = ins["scale"]       # [N, G] fp32
    b_d = ins["bias"]         # [1, N] fp32
    out_d = outs["out"]       # [M, N] fp32

    with ctx:
        const = ctx.enter_context(tc.tile_pool(name="const", bufs=1))
        wres = ctx.enter_context(tc.tile_pool(name="wres", bufs=1))
        deq = ctx.enter_context(tc.tile_pool(name="deq", bufs=2))
        xin = ctx.enter_context(tc.tile_pool(name="xin", bufs=2))
        xtp = ctx.enter_context(tc.tile_pool(name="xtp", bufs=2))
        outp = ctx.enter_context(tc.tile_pool(name="outp", bufs=3))
        psum = ctx.enter_context(tc.tile_pool(name="psum", bufs=2, space="PSUM"))

        ones_t = const.tile([1, P], BF16)
        nc.any.memset(ones_t[:], 1.0)
        bias_f = const.tile([1, N], FP32)
        nc.sync.dma_start(out=bias_f[:], in_=b_d[:])
        bias_t = const.tile([1, N], BF16)
        nc.vector.tensor_copy(bias_t[:], bias_f[:])

        # resident transposed weights: [P(k), KT, N] bf16
        wT = wres.tile([P, KT, N], BF16)

        for i in range(NT):
            mn_t = deq.tile([P, G], FP32, tag="mn")
            nc.sync.dma_start(out=mn_t[:], in_=mn_d[i * P:(i + 1) * P])
            sc_t = deq.tile([P, G], FP32, tag="sc")
            nc.sync.dma_start(out=sc_t[:], in_=sc_d[i * P:(i + 1) * P])
            inv_t = deq.tile([P, G], FP32, tag="inv")
            nc.vector.reciprocal(inv_t[:], sc_t[:])

            for gc in range(GC):
                gs_ = slice(gc * G_CH, (gc + 1) * G_CH)
                pk_t = deq.tile([P, G_CH, 32], I32, tag="pk")
                nc.sync.dma_start(out=pk_t[:], in_=pk_d[i * P:(i + 1) * P, gs_])

                vals = deq.tile([P, G_CH, GS], I32, tag="vals")
                nc.vector.tensor_scalar(
                    vals[:, :, 0:32], pk_t[:], 4, 15,
                    mybir.AluOpType.logical_shift_right,
                    mybir.AluOpType.bitwise_and)
                nc.vector.tensor_scalar(
                    vals[:, :, 32:64], pk_t[:], 15, None,
                    mybir.AluOpType.bitwise_and)

                w1 = deq.tile([P, G_CH, GS], BF16, tag="w1")
                inv_b = inv_t[:, gs_].unsqueeze(2).broadcast_to([P, G_CH, GS])
                nc.vector.tensor_tensor(w1[:], vals[:], inv_b,
                                        mybir.AluOpType.mult)
                wbf = deq.tile([P, G_CH, GS], BF16, tag="wbf")
                mn_b = mn_t[:, gs_].unsqueeze(2).broadcast_to([P, G_CH, GS])
                nc.vector.tensor_tensor(wbf[:], w1[:], mn_b,
                                        mybir.AluOpType.add)

                # [P(n), K_CH] -> wT[:, k-tile range, i*P:(i+1)*P]
                kt0 = gc * (K_CH // P)
                nc.sync.dma_start_transpose(
                    wT[:, kt0:kt0 + K_CH // P, i * P:(i + 1) * P],
                    wbf[:].rearrange("p g j -> p (g j)"))

        for m in range(MT):
            xb = xin.tile([P, K], BF16, tag="xb")
            for xc in range(K // X_CH):
                xs_ = slice(xc * X_CH, (xc + 1) * X_CH)
                xf = xin.tile([P, X_CH], FP32, tag="xf")
                nc.sync.dma_start(out=xf[:], in_=x_d[m * P:(m + 1) * P, xs_])
                nc.scalar.activation(xb[:, xs_], xf[:],
                                     mybir.ActivationFunctionType.Copy)
            xT = xtp.tile([P, KT, P], BF16, tag="xT")
            nc.sync.dma_start_transpose(xT[:], xb[:])

            for nb in range(NBS):
                pt = psum.tile([P, NB], FP32, tag="pt")
                nc.tensor.matmul(pt[:], lhsT=ones_t[:],
                                 rhs=bias_t[:, nb * NB:(nb + 1) * NB],
                                 start=True, stop=False)
                for k in range(KT):
                    nc.tensor.matmul(pt[:], lhsT=xT[:, k, :],
                                     rhs=wT[:, k, nb * NB:(nb + 1) * NB],
                                     start=False, stop=(k == KT - 1))
                ot = outp.tile([P, NB], FP32, tag="ot")
                nc.scalar.activation(ot[:], pt[:],
                                     mybir.ActivationFunctionType.Copy)
                nc.sync.dma_start(
                    out=out_d[m * P:(m + 1) * P, nb * NB:(nb + 1) * NB],
                    in_=ot[:])


_CACHED = {}


def _build():
    if "nc" in _CACHED:
        return _CACHED["nc"]
    nc = bacc.Bacc("TRN2", target_bir_lowering=False, debug=False)
    tensors = {
        "x": nc.dram_tensor("x", [M_CORE, IN], FP32, kind="ExternalInput"),
        "packed": nc.dram_tensor("packed", [N_CORE, IN // GS, GS // 2], I32,
                                 kind="ExternalInput"),
        "mn": nc.dram_tensor("mn", [N_CORE, IN // GS], FP32,
                             kind="ExternalInput"),
        "scale": nc.dram_tensor("scale", [N_CORE, IN // GS], FP32,
                                kind="ExternalInput"),
        "bias": nc.dram_tensor("bias", [1, N_CORE], FP32,
                               kind="ExternalInput"),
        "out": nc.dram_tensor("out", [M_CORE, N_CORE], FP32,
                              kind="ExternalOutput"),
    }
    ins = {k: tensors[k].ap() for k in ("x", "packed", "mn", "scale", "bias")}
    outs = {"out": tensors["out"].ap()}
    with tile.TileContext(nc) as tc:
        _emit_kernel(tc, outs, ins, M=M_CORE, K=IN, N=N_CORE, NB=NB)
    nc.compile()
    _CACHED["nc"] = nc
    return nc


def kernel(x, packed, mn, scale, bias, _trace=False, _trace_kwargs=None):
    nc = _build()

    xf = np.ascontiguousarray(x.reshape(B * S, IN).astype(np.float32))
    in_maps = []
    for r in range(R_SHARDS):
        for c in range(C_SHARDS):
            in_maps.append({
                "x": xf[r * M_CORE:(r + 1) * M_CORE],
                "packed": np.ascontiguousarray(
                    packed[c * N_CORE:(c + 1) * N_CORE]),
                "mn": np.ascontiguousarray(
                    mn[c * N_CORE:(c + 1) * N_CORE, :, 0]),
                "scale": np.ascontiguousarray(
                    scale[c * N_CORE:(c + 1) * N_CORE, :, 0]),
                "bias": np.ascontiguousarray(
                    bias[c * N_CORE:(c + 1) * N_CORE].reshape(1, N_CORE)),
            })

    res = run_bass_kernel_spmd(
        nc, in_maps, core_ids=list(range(R_SHARDS * C_SHARDS)),
        trace=_trace, **(_trace_kwargs or {}))

    out = np.empty((B * S, OUT), np.float32)
    for r in range(R_SHARDS):
        for c in range(C_SHARDS):
            shard = res.results[r * C_SHARDS + c]["out"]
            out[r * M_CORE:(r + 1) * M_CORE,
                c * N_CORE:(c + 1) * N_CORE] = shard
    kernel.last_exec_time_ns = res.exec_time_ns
    kernel.last_profile = res.profile_json
    return out.reshape(B, S, OUT)


# revision 10
# speedup vs baseline: 1.1179x; 1.1179x over previous
"""4-bit groupwise-quantized linear layer (CLinear) on 8 Trainium2 NeuronCores.

Full-input contract: kernel(**inputs) takes the unsharded numpy inputs
  x      [4, 2048, 4096] fp32
  packed [4096, 64, 32]  int32 (byte values; hi nibble = first half of each
                                quant group, lo nibble = second half)
  mn     [4096, 64, 1]   fp32
  scale  [4096, 64, 1]   fp32
  bias   [4096]          fp32
and returns out[4, 2048, 4096] fp32 = x @ dequant(packed, mn, scale).T + bias.

Sharding: 2D grid over 8 cores — 4 token-row groups x 2 out-column groups.
Core (r, c) computes out[r*2048:(r+1)*2048, c*2048:(c+1)*2048] (transposed on
device, transposed back during host assembly). No collectives.

Device kernel per core (v2 design):
  - dequantize the 2048x4096 weight shard on-chip, n-tile granular (nibble
    extraction on DVE/GPSIMD, scale/offset via broadcast APs), bf16, and
    DMA-transpose each n-tile into a resident [k, n] SBUF tile;
  - stream x in 512-token blocks: fp32->bf16 (scalar engine), DMA-transpose
    to [k, m] layout;
  - matmuls with the weight n-tile stationary and tokens moving, fp32 PSUM
    accumulation -> psum holds out.T[n-tile, tokens]; bias is a free
    per-partition add during the scalar-engine PSUM eviction.
  - n-tile-granular dependencies let the dequant pipeline overlap the first
    matmul pass; x-prep for block q+1 overlaps pass q.
"""

import sys
from contextlib import ExitStack

import numpy as np

if "/opt/trn_rl_repo" not in sys.path:
    sys.path.insert(0, "/opt/trn_rl_repo")

import concourse.mybir as mybir
import concourse.tile as tile
from concourse import bacc
from concourse.bass_utils import run_bass_kernel_spmd

FP32 = mybir.dt.float32
BF16 = mybir.dt.bfloat16
I32 = mybir.dt.int32
U8 = mybir.dt.uint8
P = 128
GS = 64  # quant group size

# problem shape (hardcoded)
B, S, IN, OUT = 4, 2048, 4096, 4096
R_SHARDS, C_SHARDS = 2, 4
M_CORE = B * S // R_SHARDS      # 2048 tokens per core
N_CORE = OUT // C_SHARDS        # 2048 out features per core
MB = 512                        # tokens per matmul block


def _emit_kernel(tc, outs, ins, M, K, N, MB=512, G_CH=16):
    nc = tc.nc
    ctx = ExitStack()
    G = K // GS
    KT = K // P
    NT = N // P
    QT = M // MB
    MT_Q = MB // P
    G_CH = min(G_CH, G)
    GC = G // G_CH
    assert K % P == 0 and N % P == 0 and M % MB == 0 and MB % P == 0

    x_d = ins["x"]            # [M, K] fp32
    pk_d = ins["packed"]      # [N, G, 32] int32
    mn_d = ins["mn"]          # [N, G] fp32
    sc_d = ins["scale"]       # [N, G] fp32
    b_d = ins["bias"]         # [1, N] fp32
    out_d = outs["out"]       # [N, M] fp32  (transposed)

    with ctx:
        const = ctx.enter_context(tc.tile_pool(name="const", bufs=1))
        wres = ctx.enter_context(tc.tile_pool(name="wres", bufs=NT))
        deq = ctx.enter_context(tc.tile_pool(name="deq", bufs=2))
        xin = ctx.enter_context(tc.tile_pool(name="xin", bufs=2))
        xbp = ctx.enter_context(tc.tile_pool(name="xbp", bufs=1))
        xtp = ctx.enter_context(tc.tile_pool(name="xtp", bufs=2))
        outp = ctx.enter_context(tc.tile_pool(name="outp", bufs=2))
        psum = ctx.enter_context(tc.tile_pool(name="psum", bufs=3, space="PSUM"))

        # bias laid out [P, NT]: column nt holds bias[nt*128:(nt+1)*128]
        bias_pt = const.tile([P, NT], FP32)
        nc.sync.dma_start(out=bias_pt[:],
                          in_=b_d[:].rearrange("1 (t p) -> p t", p=P))

        def dequant_ntile(nt, eng):
            mn_t = deq.tile([P, G], FP32, tag="mn")
            nc.sync.dma_start(out=mn_t[:], in_=mn_d[nt * P:(nt + 1) * P])
            sc_t = deq.tile([P, G], FP32, tag="sc")
            nc.sync.dma_start(out=sc_t[:], in_=sc_d[nt * P:(nt + 1) * P])
            inv_t = deq.tile([P, G], FP32, tag="inv")
            nc.vector.reciprocal(inv_t[:], sc_t[:])

            wbf = deq.tile([P, G, GS], BF16, tag="wbf")
            for gc in range(GC):
                gs_ = slice(gc * G_CH, (gc + 1) * G_CH)
                pk_t = deq.tile([P, G_CH, 32], I32, tag="pk")
                nc.sync.dma_start(out=pk_t[:],
                                  in_=pk_d[nt * P:(nt + 1) * P, gs_])
                pk8 = deq.tile([P, G_CH, 32], U8, tag="pk8")
                eng.tensor_copy(pk8[:], pk_t[:])
                vals = deq.tile([P, G_CH, GS], U8, tag="vals")
                eng.tensor_scalar(
                    vals[:, :, 0:32], pk8[:], 4, None,
                    mybir.AluOpType.logical_shift_right)
                eng.tensor_scalar(
                    vals[:, :, 32:64], pk8[:], 15, None,
                    mybir.AluOpType.bitwise_and)
                inv_b = inv_t[:, gs_].unsqueeze(2).broadcast_to([P, G_CH, GS])
                eng.tensor_tensor(wbf[:, gs_], vals[:], inv_b,
                                  mybir.AluOpType.mult)
                mn_b = mn_t[:, gs_].unsqueeze(2).broadcast_to([P, G_CH, GS])
                eng.tensor_tensor(wbf[:, gs_], wbf[:, gs_], mn_b,
                                  mybir.AluOpType.add)

            wt = wres.tile([P, KT, P], BF16, tag="wt")
            nc.sync.dma_start_transpose(
                wt[:], wbf[:].rearrange("p g j -> p (g j)"))
            return wt

        wts = []
        for nt in range(NT):
            wts.append(dequant_ntile(nt, nc.vector))

        def xprep(q, xT):
            for mt in range(MT_Q):
                m0 = q * MB + mt * P
                xb = xbp.tile([P, K], BF16, tag="xb")
                for xc in range(4):
                    xf = xin.tile([P, K // 4], FP32, tag="xf")
                    sl = slice(xc * K // 4, (xc + 1) * K // 4)
                    nc.sync.dma_start(out=xf[:], in_=x_d[m0:m0 + P, sl])
                    nc.scalar.activation(xb[:, sl], xf[:],
                                         mybir.ActivationFunctionType.Copy)
                nc.sync.dma_start_transpose(
                    xT[:, :, mt * P:(mt + 1) * P], xb[:])

        xT_cur = xtp.tile([P, KT, MB], BF16, tag="xT")
        xprep(0, xT_cur)
        for q in range(QT):
            xT_next = None
            if q + 1 < QT:
                xT_next = xtp.tile([P, KT, MB], BF16, tag="xT")
                xprep(q + 1, xT_next)
            for nt in range(NT):
                pt = psum.tile([P, MB], FP32, tag="pt")
                for k in range(KT):
                    nc.tensor.matmul(pt[:], lhsT=wts[nt][:, k, :],
                                     rhs=xT_cur[:, k, :],
                                     start=(k == 0), stop=(k == KT - 1))
                ot = outp.tile([P, MB], FP32, tag="ot")
                nc.scalar.activation(ot[:], pt[:],
                                     mybir.ActivationFunctionType.Identity,
                                     bias=bias_pt[:, nt:nt + 1])
                nc.sync.dma_start(
                    out=out_d[nt * P:(nt + 1) * P, q * MB:(q + 1) * MB],
                    in_=ot[:])
            xT_cur = xT_next


_CACHED = {}


def _build():
    if "nc" in _CACHED:
        return _CACHED["nc"]
    nc = bacc.Bacc("TRN2", target_bir_lowering=False, debug=False)
    tensors = {
        "x": nc.dram_tensor("x", [M_CORE, IN], FP32, kind="ExternalInput"),
        "packed": nc.dram_tensor("packed", [N_CORE, IN // GS, GS // 2], I32,
                                 kind="ExternalInput"),
        "mn": nc.dram_tensor("mn", [N_CORE, IN // GS], FP32,
                             kind="ExternalInput"),
        "scale": nc.dram_tensor("scale", [N_CORE, IN // GS], FP32,
                                kind="ExternalInput"),
        "bias": nc.dram_tensor("bias", [1, N_CORE], FP32,
                               kind="ExternalInput"),
        "out": nc.dram_tensor("out", [N_CORE, M_CORE], FP32,
                              kind="ExternalOutput"),
    }
    ins = {k: tensors[k].ap() for k in ("x", "packed", "mn", "scale", "bias")}
    outs = {"out": tensors["out"].ap()}
    with tile.TileContext(nc) as tc:
        _emit_kernel(tc, outs, ins, M=M_CORE, K=IN, N=N_CORE, MB=MB)
    nc.compile()
    _CACHED["nc"] = nc
    return nc


def kernel(x, packed, mn, scale, bias, _trace=False, _trace_kwargs=None):
    nc = _build()

    xf = np.ascontiguousarray(x.reshape(B * S, IN).astype(np.float32))
    in_maps = []
    for r in range(R_SHARDS):
        for c in range(C_SHARDS):
            in_maps.append({
                "x": xf[r * M_CORE:(r + 1) * M_CORE],
                "packed": np.ascontiguousarray(
                    packed[c * N_CORE:(c + 1) * N_CORE]),
                "mn": np.ascontiguousarray(
                    mn[c * N_CORE:(c + 1) * N_CORE, :, 0]),
                "scale": np.ascontiguousarray(
                    scale[c * N_CORE:(c + 1) * N_CORE, :, 0]),
                "bias": np.ascontiguousarray(
                    bias[c * N_CORE:(c + 1) * N_CORE].reshape(1, N_CORE)),
            })

    res = run_bass_kernel_spmd(
        nc, in_maps, core_ids=list(range(R_SHARDS * C_SHARDS)),
        trace=_trace, **(_trace_kwargs or {}))

    out = np.empty((B * S, OUT), np.float32)
    for r in range(R_SHARDS):
        for c in range(C_SHARDS):
            shard = res.results[r * C_SHARDS + c]["out"]  # [N_CORE, M_CORE]
            out[r * M_CORE:(r + 1) * M_CORE,
                c * N_CORE:(c + 1) * N_CORE] = shard.T
    kernel.last_exec_time_ns = res.exec_time_ns
    kernel.last_profile = res.profile_json
    return out.reshape(B, S, OUT)
